# revision 9
# baseline (speedup 1.0000x reference)
"""GCN+MLP (ChebConv K=2, sym norm) Trainium2 Bass kernel — v2.

nn_GCNMLP_81320910782821: B=32, T=12, E=10000, D=4, C=128, H=64,
N_EDGES=160000, out [B, 12, E, 4].

Strategy (data-parallel over batch, 4 b per core):
  * Host folds conv+embed into one [48,128] matrix and builds the
    aggregation operator S as a block-dense [79x79] grid of 128x128
    bf16 blocks (per-edge random access costs 100-270ns/edge on every
    gather mechanism on this part, so the "gather" runs on the PE as a
    dense block SpMM instead: ~0.8 TFLOP bf16 total, ~2ms/core).
  * Layer-1 aggregation runs in the 48-dim pre-embed space
    (S@(X48 A) == (S@X48) A), layer 2 aggregates hW = h1 @ W1 kept
    node-major resident in SBUF.
  * Self terms / MLP are channel-major PE matmuls fused per window;
    node<->channel glue via identity matmuls; outputs node-major,
    reordered on host.
"""
import sys

sys.path.insert(0, "/opt/trn_rl_repo")

import numpy as np
import ml_dtypes

B, T, E, D = 32, 12, 10000, 4
C, H = 128, 64
N_PRED, PD = 12, 4
NE = 160000
N_CORES = 8
BPC = B // N_CORES          # batches per core
WIN = 128
NW = (E + WIN - 1) // WIN   # 79 destination windows (last has 16 nodes)
EP = NW * WIN               # padded node count (10112)
G = 3                       # destination windows per PSUM group
BW = 16                     # source-window band per A-panel DMA

GROUPS = [list(range(g0, min(g0 + G, NW))) for g0 in range(0, NW, G)]
BANDS = [(b0, min(b0 + BW, NW)) for b0 in range(0, NW, BW)]

bf16 = ml_dtypes.bfloat16

_CACHE = {}


def _build_ablocks(edge_index):
    """Block-dense aggregation operator, laid out for banded streaming.

    Returns (abl, offsets, srow) where abl is flat bf16 and, for group g
    and band bi, offsets[g][bi] is the element offset of a chunk shaped
    [128 src, len(GROUPS[g]) * bw * 128] with free order
    (wd_in_group, ws_in_band, dst).
    """
    row = np.asarray(edge_index[0], dtype=np.int64)
    col = np.asarray(edge_index[1], dtype=np.int64)
    deg = np.bincount(row, minlength=E).astype(np.float32)
    dis = np.where(deg > 0, 1.0 / np.sqrt(np.maximum(deg, 1.0)), 0.0).astype(
        np.float32
    )
    w = (-dis[row] * dis[col]).astype(np.float32)
    srow = np.bincount(row, weights=w.astype(np.float64), minlength=E).astype(
        np.float32
    )

    # per-destination-window panels: [128 src, NW*128 (ws,dst)]
    panels = np.zeros((NW, 128, NW * 128), dtype=bf16)
    wd_all = row >> 7
    for wd in range(NW):
        m = wd_all == wd
        r, c_, ww = row[m], col[m], w[m]
        p = np.zeros(128 * NW * 128, np.float32)
        flat = (c_ & 127) * (NW * 128) + (c_ >> 7) * 128 + (r & 127)
        np.add.at(p, flat, ww)
        panels[wd] = p.reshape(128, NW * 128).astype(bf16)

    chunks, offsets = [], []
    off = 0
    for g, wds in enumerate(GROUPS):
        offsets.append([])
        for (b0, b1) in BANDS:
            sl = panels[wds, :, b0 * 128:b1 * 128]        # [Gg, 128, bw*128]
            ch = np.ascontiguousarray(sl.transpose(1, 0, 2))
            chunks.append(ch.reshape(128, -1))
            offsets[-1].append(off)
            off += chunks[-1].size
    abl = np.concatenate([c.reshape(-1) for c in chunks])
    return abl, offsets, srow


def _build_program(offsets):
    import concourse.bacc as bacc
    import concourse.mybir as mybir
    from concourse.tile import TileContext

    fp32 = mybir.dt.float32
    bfl = mybir.dt.bfloat16
    AF = mybir.ActivationFunctionType

    nc = bacc.Bacc("TRN2", target_bir_lowering=False, num_devices=N_CORES)

    total_abl = sum(
        128 * len(GROUPS[g]) * (b1 - b0) * 128
        for g in range(len(GROUPS)) for (b0, b1) in BANDS)
    d_abl = nc.declare_dram_parameter("abl", [total_abl], bfl, isOutput=False)
    d_x48w = nc.declare_dram_parameter("x48w", [EP, BPC * 64], bfl, isOutput=False)
    d_x48T = nc.declare_dram_parameter("x48T", [BPC, 48, E], bfl, isOutput=False)
    d_srow = nc.declare_dram_parameter("srow", [1, E], bfl, isOutput=False)
    d_ident = nc.declare_dram_parameter("ident", [128, 128], bfl, isOutput=False)
    d_identf = nc.declare_dram_parameter("identf", [128, 128], fp32, isOutput=False)
    d_a0 = nc.declare_dram_parameter("a0", [48, 128], bfl, isOutput=False)
    d_a1p = nc.declare_dram_parameter("a1p", [64, 128], bfl, isOutput=False)
    d_w0p = nc.declare_dram_parameter("w0p", [128, 128], bfl, isOutput=False)
    d_w1p = nc.declare_dram_parameter("w1p", [128, 128], bfl, isOutput=False)
    d_mw1 = nc.declare_dram_parameter("mw1", [128, 64], bfl, isOutput=False)
    d_mw2 = nc.declare_dram_parameter("mw2", [64, 48], bfl, isOutput=False)
    d_r1v = nc.declare_dram_parameter("r1v", [1, 128], bfl, isOutput=False)
    d_b1 = nc.declare_dram_parameter("b1", [128, 1], fp32, isOutput=False)
    d_b2 = nc.declare_dram_parameter("b2", [128, 1], fp32, isOutput=False)
    d_mb1 = nc.declare_dram_parameter("mb1", [64, 1], fp32, isOutput=False)
    d_mb2 = nc.declare_dram_parameter("mb2", [48, 1], fp32, isOutput=False)
    d_out = nc.declare_dram_parameter("outp", [BPC, E, 48], fp32, isOutput=True)
    d_zs2 = nc.dram_tensor("zs2", [NW, 128, BPC * 128], bfl)

    with TileContext(nc) as tc:
        with (
            tc.tile_pool(name="meta", bufs=1) as meta,
            tc.tile_pool(name="apan", bufs=2) as apan,
            tc.tile_pool(name="rhsb", bufs=2) as rhsb,
            tc.tile_pool(name="xtp", bufs=4) as xtp,
            tc.tile_pool(name="hwp", bufs=1) as hwp,
            tc.tile_pool(name="aggs", bufs=3) as aggsp,
            tc.tile_pool(name="tst", bufs=4) as tst,
            tc.tile_pool(name="h1w", bufs=2) as h1wp,
            tc.tile_pool(name="zst", bufs=3) as zstp,
            tc.tile_pool(name="h2st", bufs=2) as h2st,
            tc.tile_pool(name="zmst", bufs=2) as zmst,
            tc.tile_pool(name="prst", bufs=2) as prst,
            tc.tile_pool(name="post", bufs=3) as post,
            tc.tile_pool(name="psA", bufs=G + 1, space="PSUM") as psA,
            tc.tile_pool(name="psZ", bufs=1, space="PSUM") as psZ,
            tc.tile_pool(name="psT", bufs=1, space="PSUM") as psT,
            tc.tile_pool(name="psM", bufs=1, space="PSUM") as psM,
            tc.tile_pool(name="psN", bufs=1, space="PSUM") as psN,
        ):
            t_id = meta.tile([128, 128], bfl)
            t_idf = meta.tile([128, 128], fp32)
            t_srow = meta.tile([1, E], bfl)
            t_a0 = meta.tile([48, 128], bfl)
            t_a1p = meta.tile([64, 128], bfl)
            t_w0p = meta.tile([128, 128], bfl)
            t_w1p = meta.tile([128, 128], bfl)
            t_mw1 = meta.tile([128, 64], bfl)
            t_mw2 = meta.tile([64, 48], bfl)
            t_r1v = meta.tile([1, 128], bfl)
            t_b1 = meta.tile([128, 1], fp32)
            t_b2 = meta.tile([128, 1], fp32)
            t_mb1 = meta.tile([64, 1], fp32)
            t_mb2 = meta.tile([48, 1], fp32)
            for t, d in ((t_id, d_ident), (t_idf, d_identf), (t_srow, d_srow),
                         (t_a0, d_a0), (t_a1p, d_a1p), (t_w0p, d_w0p),
                         (t_w1p, d_w1p), (t_mw1, d_mw1), (t_mw2, d_mw2),
                         (t_r1v, d_r1v), (t_b1, d_b1), (t_b2, d_b2),
                         (t_mb1, d_mb1), (t_mb2, d_mb2)):
                nc.sync.dma_start(out=t[:], in_=d[:])

            # node-major hW = h1 @ W1', resident across L2 aggregation
            t_hw = hwp.tile([128, NW, BPC * 128], bfl)
            nc.vector.memset(t_hw[:], 0.0)

            def agg_pass(layer, elem, consume):
                """Banded block SpMM: per group of G destination windows,
                accumulate over all NW source windows, then hand each
                window's [128, elem] PSUM tile to `consume`."""
                for g, wds in enumerate(GROUPS):
                    gg = len(wds)
                    aggp = [psA.tile([128, 512], fp32, space="PSUM", tag="aggp",
                                     name=f"aggp{layer}_{wd}")
                            for wd in wds]
                    for bi, (b0, b1) in enumerate(BANDS):
                        bw = b1 - b0
                        wlen = gg * bw * 128
                        a_t = apan.tile([128, G * BW * 128], bfl, tag="a")
                        src = d_abl[offsets[g][bi]:offsets[g][bi] + 128 * wlen]
                        nc.sync.dma_start(
                            out=a_t[:, :wlen],
                            in_=src.rearrange("(p f) -> p f", f=wlen))
                        if layer == 1:
                            r_t = rhsb.tile([128, BW, 256], bfl, tag="r")
                            nc.sync.dma_start(
                                out=r_t[:, :bw, :],
                                in_=d_x48w[b0 * 128:b1 * 128, :].rearrange(
                                    "(s p) c -> p s c", p=128))
                        for wsl in range(bw):
                            ws = b0 + wsl
                            rhs = (r_t[:, wsl, :] if layer == 1
                                   else t_hw[:, ws, :])
                            for wdl in range(gg):
                                lhsT = a_t[:, (wdl * bw + wsl) * 128:
                                           (wdl * bw + wsl + 1) * 128]
                                nc.tensor.matmul(
                                    aggp[wdl][:, :elem], lhsT, rhs,
                                    start=(ws == 0), stop=(ws == NW - 1))
                    for wdl, wd in enumerate(wds):
                        consume(wd, aggp[wdl])

            # ---------------- Layer 1 ----------------
            def l1_consume(wd, aggp):
                n0 = wd * WIN
                wl = min(WIN, E - n0)
                aggs = aggsp.tile([128, 512], bfl, tag="aggs")
                nc.scalar.activation(aggs[:, :256], aggp[:, :256], AF.Copy)
                zp = psZ.tile([128, 512], fp32, space="PSUM", tag="zp")
                for b in range(BPC):
                    zsl = zp[:, b * wl:(b + 1) * wl]
                    tp = psT.tile([128, 128], fp32, space="PSUM", tag="tp")
                    nc.tensor.matmul(tp[:64, :wl], aggs[:wl, b * 64:(b + 1) * 64],
                                     t_id[:wl, :wl], start=True, stop=True)
                    ts = tst.tile([64, 128], bfl, tag="ts")
                    nc.scalar.activation(ts[:, :wl], tp[:64, :wl], AF.Copy)
                    nc.tensor.matmul(zsl, t_a1p[:], ts[:, :wl],
                                     start=True, stop=False)
                    xT = xtp.tile([48, 128], bfl, tag="xT")
                    nc.sync.dma_start(out=xT[:, :wl],
                                      in_=d_x48T[b, :, n0:n0 + wl])
                    nc.tensor.matmul(zsl, t_a0[:], xT[:, :wl],
                                     start=False, stop=False)
                    nc.tensor.matmul(zsl, t_r1v[:], t_srow[:, n0:n0 + wl],
                                     start=False, stop=True)
                h1w = h1wp.tile([128, 512], bfl, tag="h1w")
                nc.scalar.activation(h1w[:, :BPC * wl], zp[:, :BPC * wl],
                                     AF.Relu, bias=t_b1[:])
                zs2w = zstp.tile([128, 512], bfl, tag="zs2w")
                for b in range(BPC):
                    h1b = h1w[:, b * wl:(b + 1) * wl]
                    hwp_ = psT.tile([128, 128], fp32, space="PSUM", tag="tp")
                    nc.tensor.matmul(hwp_[:, :wl], t_w1p[:], h1b,
                                     start=True, stop=True)
                    hws = tst.tile([128, 128], bfl, tag="hws")
                    nc.scalar.activation(hws[:, :wl], hwp_[:, :wl], AF.Copy)
                    htp = psT.tile([128, 128], fp32, space="PSUM", tag="tp")
                    nc.tensor.matmul(htp[:wl, :], hws[:, :wl], t_id[:],
                                     start=True, stop=True)
                    nc.scalar.activation(t_hw[:wl, wd, b * 128:(b + 1) * 128],
                                         htp[:wl, :], AF.Copy)
                    zsp = psT.tile([128, 128], fp32, space="PSUM", tag="tp")
                    nc.tensor.matmul(zsp[:, :wl], t_w0p[:], h1b,
                                     start=True, stop=True)
                    nc.scalar.activation(zs2w[:, b * wl:(b + 1) * wl],
                                         zsp[:, :wl], AF.Copy)
                nc.sync.dma_start(out=d_zs2[wd, :, :BPC * wl],
                                  in_=zs2w[:, :BPC * wl])

            agg_pass(1, 256, l1_consume)

            # ---------------- Layer 2 + MLP ----------------
            def l2_consume(wd, aggp):
                n0 = wd * WIN
                wl = min(WIN, E - n0)
                aggs = aggsp.tile([128, 512], bfl, tag="aggs")
                nc.scalar.activation(aggs[:, :512], aggp[:, :512], AF.Copy)
                zs2w = zstp.tile([128, 512], bfl, tag="zs2r")
                nc.sync.dma_start(out=zs2w[:, :BPC * wl],
                                  in_=d_zs2[wd, :, :BPC * wl])
                zp = psZ.tile([128, 512], fp32, space="PSUM", tag="zp")
                for b in range(BPC):
                    zsl = zp[:, b * wl:(b + 1) * wl]
                    nc.tensor.matmul(zsl, aggs[:wl, b * 128:(b + 1) * 128],
                                     t_id[:wl, :wl], start=True, stop=False)
                    nc.tensor.matmul(zsl, t_id[:],
                                     zs2w[:, b * wl:(b + 1) * wl],
                                     start=False, stop=True)
                h2w = h2st.tile([128, 512], bfl, tag="h2w")
                nc.scalar.activation(h2w[:, :BPC * wl], zp[:, :BPC * wl],
                                     AF.Relu, bias=t_b2[:])
                m1p = psM.tile([64, 512], fp32, space="PSUM", tag="m1p")
                nc.tensor.matmul(m1p[:, :BPC * wl], t_mw1[:], h2w[:, :BPC * wl],
                                 start=True, stop=True)
                zm = zmst.tile([64, 512], bfl, tag="zm")
                nc.scalar.activation(zm[:, :BPC * wl], m1p[:, :BPC * wl],
                                     AF.Relu, bias=t_mb1[:])
                m2p = psN.tile([48, 512], fp32, space="PSUM", tag="m2p")
                nc.tensor.matmul(m2p[:, :BPC * wl], t_mw2[:], zm[:, :BPC * wl],
                                 start=True, stop=True)
                pr = prst.tile([48, 512], fp32, tag="pr")
                nc.scalar.activation(pr[:, :BPC * wl], m2p[:, :BPC * wl],
                                     AF.Identity, bias=t_mb2[:])
                for b in range(BPC):
                    ptp = psT.tile([128, 128], fp32, space="PSUM", tag="tp")
                    nc.tensor.matmul(ptp[:wl, :48], pr[:, b * wl:(b + 1) * wl],
                                     t_idf[:48, :48], start=True, stop=True)
                    ps_ = post.tile([128, 64], fp32, tag="ps_")
                    nc.scalar.activation(ps_[:wl, :48], ptp[:wl, :48], AF.Copy)
                    nc.sync.dma_start(out=d_out[b, n0:n0 + wl, :],
                                      in_=ps_[:wl, :48])

            agg_pass(2, 512, l2_consume)

    nc.finalize()
    return nc


def _fold_weights(inputs):
    f32 = np.float32
    conv_w = np.asarray(inputs["conv_w"], f32)
    conv_b = np.asarray(inputs["conv_b"], f32)
    embed_w = np.asarray(inputs["embed_w"], f32)
    embed_b = np.asarray(inputs["embed_b"], f32)
    wc48 = np.ascontiguousarray(conv_w.transpose(2, 1, 0).reshape(48, 4))
    wf = wc48 @ embed_w                                   # [48, 128]
    b_h = conv_b @ embed_w + embed_b                      # [128]
    a0 = wf @ np.asarray(inputs["cheb0_w0"], f32)
    a1 = wf @ np.asarray(inputs["cheb0_w1"], f32)
    a1p = np.zeros((64, 128), f32)
    a1p[:48] = a1
    r1v = (b_h @ np.asarray(inputs["cheb0_w1"], f32)).reshape(1, 128)
    bias1 = b_h @ np.asarray(inputs["cheb0_w0"], f32) + np.asarray(
        inputs["cheb0_b"], f32)
    return dict(
        wf=wf, a0=a0, a1p=a1p, r1v=r1v,
        bias1=bias1.reshape(128, 1),
        w0p=np.asarray(inputs["cheb1_w0"], f32),
        w1p=np.asarray(inputs["cheb1_w1"], f32),
        bias2=np.asarray(inputs["cheb1_b"], f32).reshape(128, 1),
        mw1=np.asarray(inputs["mlp_w1"], f32),
        mb1=np.asarray(inputs["mlp_b1"], f32).reshape(64, 1),
        mw2=np.asarray(inputs["mlp_w2"], f32),
        mb2=np.asarray(inputs["mlp_b2"], f32).reshape(48, 1),
    )


def prepare(**inputs):
    """Build (or fetch cached) program + per-core input maps."""
    ei = np.asarray(inputs["edge_index"])
    key = (ei.tobytes()[:4096], int(ei.sum()))
    if key not in _CACHE:
        abl, offsets, srow = _build_ablocks(ei)
        nc = _build_program(offsets)
        _CACHE[key] = (nc, abl, srow)
    nc, abl, srow = _CACHE[key]

    wts = _fold_weights(inputs)
    x = np.asarray(inputs["x"], np.float32)
    x48 = np.ascontiguousarray(x.transpose(0, 2, 1, 3)).reshape(B, E, 48)

    shared = {
        "abl": abl,
        "ident": np.eye(128, dtype=bf16),
        "identf": np.eye(128, dtype=np.float32),
        "srow": srow.astype(bf16).reshape(1, E),
        "a0": wts["a0"].astype(bf16), "a1p": wts["a1p"].astype(bf16),
        "w0p": wts["w0p"].astype(bf16), "w1p": wts["w1p"].astype(bf16),
        "mw1": wts["mw1"].astype(bf16), "mw2": wts["mw2"].astype(bf16),
        "r1v": wts["r1v"].astype(bf16),
        "b1": wts["bias1"], "b2": wts["bias2"],
        "mb1": wts["mb1"], "mb2": wts["mb2"],
    }
    in_maps = []
    for cid in range(N_CORES):
        bsl = x48[cid * BPC:(cid + 1) * BPC]              # [BPC, E, 48]
        x48T = np.ascontiguousarray(bsl.transpose(0, 2, 1)).astype(bf16)
        x48w = np.zeros((EP, BPC * 64), bf16)
        for lb in range(BPC):
            x48w[:E, lb * 64:lb * 64 + 48] = bsl[lb].astype(bf16)
        in_maps.append({"x48T": x48T, "x48w": x48w, **shared})
    return nc, in_maps


def assemble(results):
    outs = []
    for cid in range(N_CORES):
        arr = np.asarray(results[cid]["outp"], np.float32)     # [BPC, E, 48]
        outs.append(arr.reshape(BPC, E, N_PRED, PD).transpose(0, 2, 1, 3))
    return np.ascontiguousarray(np.concatenate(outs, axis=0))


def kernel(**inputs):
    from concourse.bass_utils import run_bass_kernel_spmd

    nc, in_maps = prepare(**inputs)
    res = run_bass_kernel_spmd(nc, in_maps, list(range(N_CORES)))
    return assemble(res.results)
